# revision 5
# baseline (speedup 1.0000x reference)
"""Trainium2 Bass kernel for nn_BlurLayer (B=128, 224x224x3, per-sample
rotated-line motion blur, SAME depthwise conv).

Self-contained: kernel(**inputs) -> np.ndarray. Shards the batch over 8
NeuronCores (pure data parallel: 16 samples per core), compiles + runs one
SPMD Bass program via concourse.bass_utils.run_bass_kernel_spmd, gathers
the full output.

Method: the rotated blur kernel's nonzero taps all equal 1/size and form a
digitized line. The conv is computed as G matmuls per 112-row output block:
out[r, u] += W_g[p, r] * window[p, u + 3g], where the window rows carry a
per-row horizontal shift sigma(m) and the psum columns a per-row output
shift omega(Y) (both baked host-side into the blob / unshard, so all
device access patterns are static). sigma/omega are chosen per sample by a
Bellman-Ford difference-constraint solver minimizing G (distinct shifted
column groups covering every tap) -- G lands in {1,2,3} for any angle
instead of the raw column span. Masks W_g are arbitrary per-sample 0/1
matrices (block-edge clipping implements vertical SAME padding). The image
is pre-scaled by 1/size and split into fp8e4m3 hi/lo planes so matmuls run
in DoubleRow perf mode and PSUM already holds the final values (plain
copies, no scale path). Input blobs stream on two DMA rings (sync+scalar),
band-trimmed outputs on two more (gpsimd+sync); a post-compile pass drops
back-to-back reloads of identical PE weights.
"""

import math

import numpy as np
import ml_dtypes

MAXK = 32
H = W = 224
C = 3
WC = W * C                  # 672
PAD_LO = (MAXK - 1) // 2    # 15
PIMG_PAD = 800              # left zero margin (elems) of padded image row
PIMG_W = PIMG_PAD + WC + 800
RB = 56                     # out-DMA row band height

FP8 = ml_dtypes.float8_e4m3


def dedupe_ldweights(nc):
    """Replace an InstLdweights whose weights AP is identical to the
    immediately-preceding one (per block) with an InstNoOp carrying its
    sync_info: the PE array still holds those weights, so the reload is
    pure Tensor-queue overhead."""
    import concourse.mybir as mybir
    n = 0
    for fn in nc.m.functions:
        for blk in fn.blocks:
            prev_key = None
            new_insts = []
            for inst in blk.instructions:
                if isinstance(inst, mybir.InstLdweights):
                    key = (repr(inst.ins), repr(getattr(inst, "perf_mode", None)),
                           repr(getattr(inst, "is_transpose", None)))
                    if key == prev_key:
                        n += 1
                        inst = mybir.InstNoOp(
                            name=f"{inst.name}-ldwdedup",
                            engine=inst.engine,
                            ins=[], outs=[],
                            sync_info=inst.sync_info,
                        )
                    else:
                        prev_key = key
                new_insts.append(inst)
            blk.instructions = new_insts
    return n


# ---------------------------------------------------------------- host math
def rotate_nearest_np(img, rad):
    K = img.shape[0]
    cos, sin = np.cos(rad), np.sin(rad)
    coords = np.arange(K, dtype=np.float32)
    yy, xx = np.meshgrid(coords, coords, indexing="ij")
    e = np.float32(K - 1)
    x_off = (e - (cos * e - sin * e)) * 0.5
    y_off = (e - (sin * e + cos * e)) * 0.5
    sx = cos * xx - sin * yy + x_off
    sy = sin * xx + cos * yy + y_off
    ix = np.round(sx).astype(np.int32)
    iy = np.round(sy).astype(np.int32)
    valid = (ix >= 0) & (ix < K) & (iy >= 0) & (iy < K)
    g = img[np.clip(iy, 0, K - 1), np.clip(ix, 0, K - 1)]
    return np.where(valid, g, np.float32(0.0))


def tap_pairs(ker):
    """Valid (m, Y, kx) triples: img row m = Y + ky - 15 for out row Y and
    tap (ky, kx), both m and Y in [0, 224)."""
    ys, xs = np.nonzero(ker)
    Y = np.arange(H)
    M, YY, KX = [], [], []
    for ky, kx in zip(ys, xs):
        m = Y + ky - 15
        ok = (m >= 0) & (m < H)
        M.append(m[ok])
        YY.append(Y[ok])
        KX.append(np.full(int(ok.sum()), kx))
    return np.concatenate(M), np.concatenate(YY), np.concatenate(KX)


def solve_shear(ker, max_rounds=400):
    """Choose integer shift profiles sigma (img rows) / nu (out rows)
    minimizing G = #values of g = kx - sigma[m] + nu[Y] over all taps.
    Difference-constraint feasibility via vectorized Bellman-Ford.
    Returns (G, sigma[224], nu[224], c) with g - c in [0, G)."""
    M, Y, KX = tap_pairs(ker)
    key = M * H + Y
    order = np.argsort(key)
    key_s, kx_s = key[order], KX[order]
    uk, idx = np.unique(key_s, return_index=True)
    lo = np.minimum.reduceat(kx_s, idx)
    hi = np.maximum.reduceat(kx_s, idx)
    um, uy = uk // H, uk % H
    G0 = int((hi - lo).max()) + 1
    for G in range(G0, 40):
        src = np.concatenate([224 + uy, um])
        dst = np.concatenate([um, 224 + uy])
        w = np.concatenate([lo, (G - 1) - hi]).astype(np.int64)
        dist = np.zeros(448, np.int64)
        ok = False
        for _ in range(max_rounds):
            nd = dist.copy()
            np.minimum.at(nd, dst, dist[src] + w)
            if np.array_equal(nd, dist):
                ok = True
                break
            dist = nd
        if ok:
            sigma, nu = dist[:224], dist[224:]
            g = KX - sigma[M] + nu[Y]
            c = int(g.min())
            assert int(g.max()) - c + 1 <= G
            return int(g.max()) - c + 1, sigma, nu, c
    raise RuntimeError("no feasible G")


def sample_plan(tbl_ch0, amt_b, ang_b):
    """-> dict with scale, G, sigma, nu, c, wout, transposed, ker."""
    rad = np.float32(ang_b * math.pi / 180.0)
    ker0 = rotate_nearest_np(tbl_ch0[amt_b], rad)
    ys, xs = np.nonzero(ker0)
    scale = float(ker0[ys[0], xs[0]])
    best = None
    for tr in (False, True):
        km = ker0.T if tr else ker0
        G, sigma, nu, c = solve_shear(km)
        w0 = int(nu[0:112].max() - nu[0:112].min())
        w1 = int(nu[112:224].max() - nu[112:224].min())
        wout = WC + 3 * max(w0, w1)
        cost = 2 * G * wout
        if best is None or cost < best["cost"]:
            best = dict(cost=cost, G=G, sigma=sigma, nu=nu, c=c, wout=wout,
                        transposed=tr, ker=km, scale=np.float32(scale))
    return best


def prepare_host(x, kernels_table, amt, angles, n_cores=8):
    B = x.shape[0]
    assert B % n_cores == 0
    slots = B // n_cores
    tbl_ch0 = np.ascontiguousarray(kernels_table[:, :, :, 0])

    plans = [sample_plan(tbl_ch0, int(amt[b]), int(angles[b]))
             for b in range(B)]
    Gs = np.array([p["G"] for p in plans])
    wos = np.array([p["wout"] for p in plans])

    # slot packing: G uniform-ish per slot (sort by G then wout, rows of 8)
    order = np.lexsort((-wos, -Gs))
    asg = order.reshape(slots, n_cores)

    slotG = np.array([Gs[asg[j]].max() for j in range(slots)])
    slotW = np.array([wos[asg[j]].max() for j in range(slots)])

    # schedule: a light slot first (its input DMA completes fastest, so
    # the PE starts early), the lightest last (small tail), heavy middle.
    slot_cost = 2 * slotG * slotW
    o = np.argsort(-slot_cost, kind="stable")   # heavy .. light
    sched = np.concatenate([[o[-2]], o[:-2], [o[-1]]])
    asg = asg[sched]
    slotG = slotG[sched]
    slotW = slotW[sched]

    gmax = slotG
    wout = slotW
    wprime = ((wout + 3 * gmax + 7) // 8) * 8
    blobw = 4 * wprime + 224 * gmax            # 4 image planes + mask table
    col_base = np.concatenate([[0], np.cumsum(blobw)])[:-1]
    totbw = int(blobw.sum())

    # per-sample omega profiles (before blob build, to size the out bands)
    # omega[j, cidx, hb, r] = numax_hb - nu[R+r]
    omega = np.zeros((slots, n_cores, 2, 112), np.int64)
    for j in range(slots):
        for cidx in range(n_cores):
            p = plans[asg[j, cidx]]
            for hb, R in enumerate((0, 112)):
                nub = p["nu"][R:R + 112]
                omega[j, cidx, hb] = int(nub.max()) - nub

    # out bands: rows [0,56) and [56,112) per hb; per (slot, hb, band):
    # U0 = min 3*omega over slot samples+band rows, bw = max(3*omega)+WC-U0
    nb = 112 // RB
    U0 = np.zeros((slots, 2, nb), np.int64)
    bw = np.zeros((slots, 2, nb), np.int64)
    for j in range(slots):
        for hb in range(2):
            for bd in range(nb):
                om = 3 * omega[j, :, hb, bd * RB:(bd + 1) * RB]
                U0[j, hb, bd] = om.min()
                bw[j, hb, bd] = om.max() + WC - U0[j, hb, bd]
                assert U0[j, hb, bd] + bw[j, hb, bd] <= wout[j]
    obase = np.zeros((slots, 2, nb), np.int64)
    acc = 0
    for j in range(slots):
        for hb in range(2):
            for bd in range(nb):
                obase[j, hb, bd] = acc
                acc += RB * int(bw[j, hb, bd])
    totout = int(acc)

    in_maps = []
    mapping = np.zeros((n_cores, slots), np.int64)
    for cidx in range(n_cores):
        blob = np.zeros((128, totbw), FP8)
        for j in range(slots):
            b = int(asg[j, cidx])
            p = plans[b]
            G = int(gmax[j])
            Wp = int(wprime[j])
            base = int(col_base[j])
            mapping[cidx, j] = b
            sigma, nu, c = p["sigma"], p["nu"], p["c"]

            # fold 1/size = 2^-k * s' : 2^-k goes into the masks (exact in
            # fp8), s' in (0.5, 1] scales the image (no dynamic-range loss)
            size = int(round(1.0 / float(p["scale"])))
            k2 = int(math.floor(math.log2(size)))
            xs_ = x[b] * np.float32(2.0 ** k2 / size)
            if p["transposed"]:
                xs_ = xs_.transpose(1, 0, 2)
            xhi = xs_.astype(FP8)
            xlo = (xs_ - xhi.astype(np.float32)).astype(FP8)
            phi = np.zeros((H, PIMG_W), FP8)
            plo = np.zeros((H, PIMG_W), FP8)
            phi[:, PIMG_PAD:PIMG_PAD + WC] = xhi.reshape(H, WC)
            plo[:, PIMG_PAD:PIMG_PAD + WC] = xlo.reshape(H, WC)

            # blob layout: [hb0_hi | hb1_hi | hb0_lo | hb1_lo | masks]
            for hb, (R, S) in enumerate(((0, 0), (112, 96))):
                numax = int(nu[R:R + 112].max())
                V0 = PIMG_PAD + 3 * (c - PAD_LO - numax)
                rows = np.arange(128)
                cols = V0 + 3 * sigma[S + rows]
                assert cols.min() >= 0 and cols.max() + Wp <= PIMG_W, \
                    (b, hb, cols.min(), cols.max(), Wp)
                for pl, pimg in enumerate((phi, plo)):
                    dst = base + (2 * pl + hb) * Wp
                    win = np.zeros((128, Wp), FP8)
                    for pp in range(128):
                        win[pp] = pimg[S + pp, cols[pp]:cols[pp] + Wp]
                    blob[:, dst:dst + Wp] = win

            # mask table: [128, G, 2(hb), 112] fp8
            wtb = base + 4 * Wp
            wcols = np.zeros((128, G, 2, 112), np.float32)
            ys, xs2 = np.nonzero(p["ker"])
            r = np.arange(112)
            for ky, kx in zip(ys, xs2):
                for hb, (R, S) in enumerate(((0, 0), (112, 96))):
                    m = R + r + ky - PAD_LO
                    pr = m - S
                    ok = (pr >= 0) & (pr < 128) & (m >= 0) & (m < H)
                    if not ok.any():
                        continue
                    g = (kx - sigma[m[ok]] + nu[R + r[ok]]) - c
                    assert g.min() >= 0 and g.max() < G, (b, hb, g.min(), g.max(), G)
                    wcols[pr[ok], g, hb, r[ok]] = 2.0 ** (-k2)
            blob[:, wtb:wtb + 224 * G] = np.ascontiguousarray(wcols).reshape(
                128, 224 * G).astype(FP8)
        in_maps.append({"ximg": blob})

    meta = {
        "slots": slots,
        "gmax": [int(v) for v in gmax],
        "wout": [int(v) for v in wout],
        "wprime": [int(v) for v in wprime],
        "blobw": [int(v) for v in blobw],
        "col_base": [int(v) for v in col_base],
        "totbw": totbw,
        "totout": totout,
        "U0": U0, "bw": bw, "obase": obase,
        "mapping": mapping,
        "omega": omega,
        "transposed": np.array([p["transposed"] for p in plans]),
    }
    return meta, in_maps


def _chunks(wout):
    """Split a result width into <=512-col PSUM chunks."""
    n = -(-wout // 512)
    w = -(-wout // n)
    out = []
    off = 0
    while off < wout:
        cc = min(w, wout - off)
        out.append((off, cc))
        off += cc
    return out


# ---------------------------------------------------------------- device IR
def build_program(meta):
    import concourse.bacc as bacc
    import concourse.mybir as mybir
    from concourse.tile import TileContext
    from bass_rust import VecI64Pair

    fp8 = mybir.dt.float8e4
    slots = meta["slots"]
    nb = 112 // RB

    nc = bacc.Bacc("TRN2")
    ximg = nc.dram_tensor("ximg", [128, meta["totbw"]], fp8, kind="ExternalInput")
    out = nc.dram_tensor("out", [1, meta["totout"]], mybir.dt.float16,
                         kind="ExternalOutput")

    def strided(tile, dims, offset):
        ap = tile[:, 0:1].copy()
        ap.ap = VecI64Pair(dims)
        ap.offset = offset
        return ap

    with TileContext(nc) as tc:
        with tc.tile_pool(name="img", bufs=10) as ipool, \
             tc.tile_pool(name="res", bufs=10) as rpool, \
             tc.tile_pool(name="ps0", bufs=2, space="PSUM") as pw0, \
             tc.tile_pool(name="ps1", bufs=2, space="PSUM") as pw1, \
             tc.tile_pool(name="ps2", bufs=2, space="PSUM") as pw2:

            wpools = [pw0, pw1, pw2]
            for j in range(slots):
                G = meta["gmax"][j]
                WO = meta["wout"][j]
                Wp = meta["wprime"][j]
                BW = meta["blobw"][j]
                base = meta["col_base"][j]
                ch = _chunks(WO)
                assert len(ch) <= 3
                blob = ipool.tile([128, BW], fp8, tag="blob", name="blob")
                wtb = 4 * Wp
                # hi windows on the sync ring, lo windows + masks on scalar
                nc.sync.dma_start(out=blob[:, 0:2 * Wp],
                                  in_=ximg[:, base:base + 2 * Wp])
                nc.scalar.dma_start(out=blob[:, 2 * Wp:BW],
                                    in_=ximg[:, base + 2 * Wp:base + BW])

                for hb in (0, 1):
                    rt = rpool.tile([112, WO], mybir.dt.float16, tag=f"rt{hb}",
                                    name=f"rt{hb}")
                    psums = [wpools[wh].tile([112, ch[wh][1]], mybir.dt.float32,
                                             tag=f"ps{wh}", name=f"ps{wh}")
                             for wh in range(len(ch))]
                    for g in range(G):
                        # same mask for both fp8 planes (hi, lo)
                        lhs = strided(blob, [[BW, 128], [0, 2], [1, 112]],
                                      wtb + 224 * g + 112 * hb)
                        for wh in range(len(ch)):
                            # planes (hi, lo) of window hb at column shift 3g
                            rhs = strided(
                                blob, [[BW, 128], [2 * Wp, 2], [1, ch[wh][1]]],
                                hb * Wp + 3 * g + ch[wh][0])
                            nc.tensor.matmul(
                                psums[wh], lhsT=lhs, rhs=rhs,
                                start=(g == 0), stop=(g == G - 1),
                                perf_mode=mybir.MatmulPerfMode.DoubleRow)
                    for wh in range(len(ch)):
                        dstc = rt[:, ch[wh][0]:ch[wh][0] + ch[wh][1]]
                        if wh == 0:
                            nc.scalar.activation(
                                out=dstc, in_=psums[wh],
                                func=mybir.ActivationFunctionType.Copy)
                        else:
                            nc.vector.tensor_copy(out=dstc, in_=psums[wh])
                    for bd in range(nb):
                        u0 = int(meta["U0"][j, hb, bd])
                        bwd = int(meta["bw"][j, hb, bd])
                        ob = int(meta["obase"][j, hb, bd])
                        src = rt[bd * RB:(bd + 1) * RB, u0:u0 + bwd]
                        dst = out[0, ob:ob + RB * bwd]
                        eng = nc.sync if (hb == 1 and bd == 1) else nc.gpsimd
                        eng.dma_start(out=dst, in_=src)
    return nc


def run_cores(meta, in_maps, trace=False):
    from concourse.bass_utils import run_bass_kernel_spmd

    nc = build_program(meta)
    nc.compile()
    dedupe_ldweights(nc)
    res = run_bass_kernel_spmd(nc, in_maps, core_ids=list(range(len(in_maps))),
                               trace=trace)
    return res


def unshard(meta, results):
    B = meta["mapping"].size
    nb = 112 // RB
    out = np.zeros((B, H, W, C), np.float32)
    for cidx, r in enumerate(results):
        o = np.asarray(r["out"], np.float32).reshape(-1)
        for j in range(meta["slots"]):
            b = meta["mapping"][cidx, j]
            img = np.zeros((H, WC), np.float32)
            for hb in range(2):
                for bd in range(nb):
                    u0 = int(meta["U0"][j, hb, bd])
                    bwd = int(meta["bw"][j, hb, bd])
                    ob = int(meta["obase"][j, hb, bd])
                    t = o[ob:ob + RB * bwd].reshape(RB, bwd)
                    om = meta["omega"][j, cidx, hb]
                    for rr in range(RB):
                        r_ = bd * RB + rr
                        u = 3 * int(om[r_]) - u0
                        img[112 * hb + r_] = t[rr, u:u + WC]
            img = img.reshape(H, W, C)
            if meta["transposed"][b]:
                img = img.transpose(1, 0, 2)
            out[b] = img
    return out


def kernel(x, kernels_table, amt, angles):
    x = np.asarray(x, np.float32)
    kernels_table = np.asarray(kernels_table, np.float32)
    amt = np.asarray(amt)
    angles = np.asarray(angles)
    meta, in_maps = prepare_host(x, kernels_table, amt, angles)
    res = run_cores(meta, in_maps)
    return unshard(meta, res.results)


# revision 8
# speedup vs baseline: 1.0209x; 1.0209x over previous
"""Trainium2 Bass kernel for nn_BlurLayer (B=128, 224x224x3, per-sample
rotated-line motion blur, SAME depthwise conv).

Self-contained: kernel(**inputs) -> np.ndarray. Shards the batch over 8
NeuronCores (pure data parallel: 16 samples per core), compiles + runs one
SPMD Bass program via concourse.bass_utils.run_bass_kernel_spmd, gathers
the full output.

Method: the rotated blur kernel's nonzero taps all equal 1/size and form a
digitized line. The conv is computed as G matmuls per 112-row output block:
out[r, u] += W_g[p, r] * window[p, u + 3g], where the window rows carry a
per-row horizontal shift sigma(m) and the psum columns a per-row output
shift omega(Y) (both baked host-side into the blob / unshard, so all
device access patterns are static). sigma/omega are chosen per sample by a
Bellman-Ford difference-constraint solver minimizing G (distinct shifted
column groups covering every tap) -- G lands in {1,2,3} for any angle
instead of the raw column span. Masks W_g are arbitrary per-sample 0/1
matrices (block-edge clipping implements vertical SAME padding). The image
is pre-scaled by 1/size and split into fp8e4m3 hi/lo planes so matmuls run
in DoubleRow perf mode and PSUM already holds the final values (plain
copies, no scale path). Input blobs stream on two DMA rings (sync+scalar),
band-trimmed outputs on two more (gpsimd+sync); a post-compile pass drops
back-to-back reloads of identical PE weights.
"""

import math

import numpy as np
import ml_dtypes

MAXK = 32
H = W = 224
C = 3
WC = W * C                  # 672
PAD_LO = (MAXK - 1) // 2    # 15
PIMG_PAD = 800              # left zero margin (elems) of padded image row
PIMG_W = PIMG_PAD + WC + 800
RB = 56                     # out-DMA row band height

FP8 = ml_dtypes.float8_e4m3


def dedupe_ldweights(nc):
    """Replace an InstLdweights whose weights AP is identical to the
    immediately-preceding one (per block) with an InstNoOp carrying its
    sync_info: the PE array still holds those weights, so the reload is
    pure Tensor-queue overhead."""
    import concourse.mybir as mybir
    n = 0
    for fn in nc.m.functions:
        for blk in fn.blocks:
            prev_key = None
            new_insts = []
            for inst in blk.instructions:
                if isinstance(inst, mybir.InstLdweights):
                    key = (repr(inst.ins), repr(getattr(inst, "perf_mode", None)),
                           repr(getattr(inst, "is_transpose", None)))
                    if key == prev_key:
                        n += 1
                        inst = mybir.InstNoOp(
                            name=f"{inst.name}-ldwdedup",
                            engine=inst.engine,
                            ins=[], outs=[],
                            sync_info=inst.sync_info,
                        )
                    else:
                        prev_key = key
                new_insts.append(inst)
            blk.instructions = new_insts
    return n


# ---------------------------------------------------------------- host math
def rotate_nearest_np(img, rad):
    K = img.shape[0]
    cos, sin = np.cos(rad), np.sin(rad)
    coords = np.arange(K, dtype=np.float32)
    yy, xx = np.meshgrid(coords, coords, indexing="ij")
    e = np.float32(K - 1)
    x_off = (e - (cos * e - sin * e)) * 0.5
    y_off = (e - (sin * e + cos * e)) * 0.5
    sx = cos * xx - sin * yy + x_off
    sy = sin * xx + cos * yy + y_off
    ix = np.round(sx).astype(np.int32)
    iy = np.round(sy).astype(np.int32)
    valid = (ix >= 0) & (ix < K) & (iy >= 0) & (iy < K)
    g = img[np.clip(iy, 0, K - 1), np.clip(ix, 0, K - 1)]
    return np.where(valid, g, np.float32(0.0))


def tap_pairs(ker):
    """Valid (m, Y, kx) triples: img row m = Y + ky - 15 for out row Y and
    tap (ky, kx), both m and Y in [0, 224)."""
    ys, xs = np.nonzero(ker)
    Y = np.arange(H)
    M, YY, KX = [], [], []
    for ky, kx in zip(ys, xs):
        m = Y + ky - 15
        ok = (m >= 0) & (m < H)
        M.append(m[ok])
        YY.append(Y[ok])
        KX.append(np.full(int(ok.sum()), kx))
    return np.concatenate(M), np.concatenate(YY), np.concatenate(KX)


def solve_shear(ker, max_rounds=400):
    """Choose integer shift profiles sigma (img rows) / nu (out rows)
    minimizing G = #values of g = kx - sigma[m] + nu[Y] over all taps.
    Difference-constraint feasibility via vectorized Bellman-Ford.
    Returns (G, sigma[224], nu[224], c) with g - c in [0, G)."""
    M, Y, KX = tap_pairs(ker)
    key = M * H + Y
    order = np.argsort(key)
    key_s, kx_s = key[order], KX[order]
    uk, idx = np.unique(key_s, return_index=True)
    lo = np.minimum.reduceat(kx_s, idx)
    hi = np.maximum.reduceat(kx_s, idx)
    um, uy = uk // H, uk % H
    G0 = int((hi - lo).max()) + 1
    for G in range(G0, 40):
        src = np.concatenate([224 + uy, um])
        dst = np.concatenate([um, 224 + uy])
        w = np.concatenate([lo, (G - 1) - hi]).astype(np.int64)
        dist = np.zeros(448, np.int64)
        ok = False
        for _ in range(max_rounds):
            nd = dist.copy()
            np.minimum.at(nd, dst, dist[src] + w)
            if np.array_equal(nd, dist):
                ok = True
                break
            dist = nd
        if ok:
            sigma, nu = dist[:224], dist[224:]
            g = KX - sigma[M] + nu[Y]
            c = int(g.min())
            assert int(g.max()) - c + 1 <= G
            return int(g.max()) - c + 1, sigma, nu, c
    raise RuntimeError("no feasible G")


def sample_plan(tbl_ch0, amt_b, ang_b):
    """-> dict with scale, G, sigma, nu, c, wout, transposed, ker."""
    rad = np.float32(ang_b * math.pi / 180.0)
    ker0 = rotate_nearest_np(tbl_ch0[amt_b], rad)
    ys, xs = np.nonzero(ker0)
    scale = float(ker0[ys[0], xs[0]])
    best = None
    for tr in (False, True):
        km = ker0.T if tr else ker0
        G, sigma, nu, c = solve_shear(km)
        w0 = int(nu[0:112].max() - nu[0:112].min())
        w1 = int(nu[112:224].max() - nu[112:224].min())
        wout = WC + 3 * max(w0, w1)
        cost = 2 * G * wout
        if best is None or cost < best["cost"]:
            best = dict(cost=cost, G=G, sigma=sigma, nu=nu, c=c, wout=wout,
                        transposed=tr, ker=km, scale=np.float32(scale))
    return best


def prepare_host(x, kernels_table, amt, angles, n_cores=8):
    B = x.shape[0]
    assert B % n_cores == 0
    slots = B // n_cores
    tbl_ch0 = np.ascontiguousarray(kernels_table[:, :, :, 0])

    plans = [sample_plan(tbl_ch0, int(amt[b]), int(angles[b]))
             for b in range(B)]
    Gs = np.array([p["G"] for p in plans])
    wos = np.array([p["wout"] for p in plans])

    # slot packing: G uniform-ish per slot (sort by G then wout, rows of 8)
    order = np.lexsort((-wos, -Gs))
    asg = order.reshape(slots, n_cores)

    slotG = np.array([Gs[asg[j]].max() for j in range(slots)])
    slotW = np.array([wos[asg[j]].max() for j in range(slots)])

    # schedule: a light slot first (its input DMA completes fastest, so
    # the PE starts early), the lightest last (small tail), heavy middle.
    slot_cost = 2 * slotG * slotW
    o = np.argsort(-slot_cost, kind="stable")   # heavy .. light
    sched = np.concatenate([[o[-2]], o[:-2], [o[-1]]])
    asg = asg[sched]
    slotG = slotG[sched]
    slotW = slotW[sched]

    gmax = slotG
    wout = slotW
    wprime = ((wout + 3 * gmax + 7) // 8) * 8
    blobw = 4 * wprime + 224 * gmax            # 4 image planes + mask table
    col_base = np.concatenate([[0], np.cumsum(blobw)])[:-1]
    totbw = int(blobw.sum())

    # per-sample omega profiles (before blob build, to size the out bands)
    # omega[j, cidx, hb, r] = numax_hb - nu[R+r]
    omega = np.zeros((slots, n_cores, 2, 112), np.int64)
    for j in range(slots):
        for cidx in range(n_cores):
            p = plans[asg[j, cidx]]
            for hb, R in enumerate((0, 112)):
                nub = p["nu"][R:R + 112]
                omega[j, cidx, hb] = int(nub.max()) - nub

    # out bands: rows [0,56) and [56,112) per hb; per (slot, hb, band):
    # U0 = min 3*omega over slot samples+band rows, bw = max(3*omega)+WC-U0
    nb = 112 // RB
    U0 = np.zeros((slots, 2, nb), np.int64)
    bw = np.zeros((slots, 2, nb), np.int64)
    for j in range(slots):
        for hb in range(2):
            for bd in range(nb):
                om = 3 * omega[j, :, hb, bd * RB:(bd + 1) * RB]
                U0[j, hb, bd] = om.min()
                bw[j, hb, bd] = om.max() + WC - U0[j, hb, bd]
                assert U0[j, hb, bd] + bw[j, hb, bd] <= wout[j]
    obase = np.zeros((slots, 2, nb), np.int64)
    acc = 0
    for j in range(slots):
        for hb in range(2):
            for bd in range(nb):
                obase[j, hb, bd] = acc
                acc += RB * int(bw[j, hb, bd])
    totout = int(acc)

    in_maps = []
    mapping = np.zeros((n_cores, slots), np.int64)
    for cidx in range(n_cores):
        blob = np.zeros((128, totbw), FP8)
        for j in range(slots):
            b = int(asg[j, cidx])
            p = plans[b]
            G = int(gmax[j])
            Wp = int(wprime[j])
            base = int(col_base[j])
            mapping[cidx, j] = b
            sigma, nu, c = p["sigma"], p["nu"], p["c"]

            # fold 1/size = 2^-k * s' : 2^-k goes into the masks (exact in
            # fp8), s' in (0.5, 1] scales the image (no dynamic-range loss)
            size = int(round(1.0 / float(p["scale"])))
            k2 = int(math.floor(math.log2(size)))
            xs_ = x[b] * np.float32(2.0 ** k2 / size)
            if p["transposed"]:
                xs_ = xs_.transpose(1, 0, 2)
            xhi = xs_.astype(FP8)
            xlo = (xs_ - xhi.astype(np.float32)).astype(FP8)
            phi = np.zeros((H, PIMG_W), FP8)
            plo = np.zeros((H, PIMG_W), FP8)
            phi[:, PIMG_PAD:PIMG_PAD + WC] = xhi.reshape(H, WC)
            plo[:, PIMG_PAD:PIMG_PAD + WC] = xlo.reshape(H, WC)

            # blob layout: [hb0_hi | hb1_hi | hb0_lo | hb1_lo | masks]
            for hb, (R, S) in enumerate(((0, 0), (112, 96))):
                numax = int(nu[R:R + 112].max())
                V0 = PIMG_PAD + 3 * (c - PAD_LO - numax)
                rows = np.arange(128)
                cols = V0 + 3 * sigma[S + rows]
                assert cols.min() >= 0 and cols.max() + Wp <= PIMG_W, \
                    (b, hb, cols.min(), cols.max(), Wp)
                for pl, pimg in enumerate((phi, plo)):
                    dst = base + (2 * pl + hb) * Wp
                    win = np.zeros((128, Wp), FP8)
                    for pp in range(128):
                        win[pp] = pimg[S + pp, cols[pp]:cols[pp] + Wp]
                    blob[:, dst:dst + Wp] = win

            # mask table: [128, G, 2(hb), 112] fp8
            wtb = base + 4 * Wp
            wcols = np.zeros((128, G, 2, 112), np.float32)
            ys, xs2 = np.nonzero(p["ker"])
            r = np.arange(112)
            for ky, kx in zip(ys, xs2):
                for hb, (R, S) in enumerate(((0, 0), (112, 96))):
                    m = R + r + ky - PAD_LO
                    pr = m - S
                    ok = (pr >= 0) & (pr < 128) & (m >= 0) & (m < H)
                    if not ok.any():
                        continue
                    g = (kx - sigma[m[ok]] + nu[R + r[ok]]) - c
                    assert g.min() >= 0 and g.max() < G, (b, hb, g.min(), g.max(), G)
                    wcols[pr[ok], g, hb, r[ok]] = 2.0 ** (-k2)
            blob[:, wtb:wtb + 224 * G] = np.ascontiguousarray(wcols).reshape(
                128, 224 * G).astype(FP8)
        in_maps.append({"ximg": blob})

    meta = {
        "slots": slots,
        "gmax": [int(v) for v in gmax],
        "wout": [int(v) for v in wout],
        "wprime": [int(v) for v in wprime],
        "blobw": [int(v) for v in blobw],
        "col_base": [int(v) for v in col_base],
        "totbw": totbw,
        "totout": totout,
        "U0": U0, "bw": bw, "obase": obase,
        "mapping": mapping,
        "omega": omega,
        "transposed": np.array([p["transposed"] for p in plans]),
    }
    return meta, in_maps


def _chunks(wout):
    """Split a result width into <=512-col PSUM chunks."""
    n = -(-wout // 512)
    w = -(-wout // n)
    out = []
    off = 0
    while off < wout:
        cc = min(w, wout - off)
        out.append((off, cc))
        off += cc
    return out


# ---------------------------------------------------------------- device IR
def build_program(meta):
    import concourse.bacc as bacc
    import concourse.mybir as mybir
    from concourse.tile import TileContext
    from bass_rust import VecI64Pair

    fp8 = mybir.dt.float8e4
    slots = meta["slots"]
    nb = 112 // RB

    nc = bacc.Bacc("TRN2")
    ximg = nc.dram_tensor("ximg", [128, meta["totbw"]], fp8, kind="ExternalInput")
    out = nc.dram_tensor("out", [1, meta["totout"]], mybir.dt.float16,
                         kind="ExternalOutput")

    def strided(tile, dims, offset):
        ap = tile[:, 0:1].copy()
        ap.ap = VecI64Pair(dims)
        ap.offset = offset
        return ap

    with TileContext(nc) as tc:
        with tc.tile_pool(name="img", bufs=slots) as ipool, \
             tc.tile_pool(name="res", bufs=10) as rpool, \
             tc.tile_pool(name="warm", bufs=1) as wmpool, \
             tc.tile_pool(name="ps0", bufs=2, space="PSUM") as pw0, \
             tc.tile_pool(name="ps1", bufs=2, space="PSUM") as pw1, \
             tc.tile_pool(name="ps2", bufs=2, space="PSUM") as pw2, \
             tc.tile_pool(name="psw", bufs=1, space="PSUM") as pww:

            # PE warmup: ~7 junk matmuls keep the PE busy through the HAM
            # activity window while the first blobs stream in, so the real
            # matmuls start at 2.4 GHz instead of 1.2.
            wm = wmpool.tile([128, 624], fp8)
            nc.vector.memset(wm, 0.0)
            wps = pww.tile([112, 512], mybir.dt.float32)
            for _ in range(7):
                nc.tensor.matmul(wps, lhsT=wm[:, 0:112], rhs=wm[:, 112:624],
                                 start=True, stop=True)

            wpools = [pw0, pw1, pw2]
            for j in range(slots):
                G = meta["gmax"][j]
                WO = meta["wout"][j]
                Wp = meta["wprime"][j]
                BW = meta["blobw"][j]
                base = meta["col_base"][j]
                ch = _chunks(WO)
                assert len(ch) <= 3
                blob = ipool.tile([128, BW], fp8, tag="blob", name="blob")
                wtb = 4 * Wp
                # hi windows on the sync ring, lo windows + masks on scalar;
                # high priority so prefetch issues ahead of dependent work
                with tc.high_priority():
                    nc.sync.dma_start(out=blob[:, 0:2 * Wp],
                                      in_=ximg[:, base:base + 2 * Wp])
                    nc.scalar.dma_start(out=blob[:, 2 * Wp:BW],
                                        in_=ximg[:, base + 2 * Wp:base + BW])

                for hb in (0, 1):
                    rt = rpool.tile([112, WO], mybir.dt.float16, tag=f"rt{hb}",
                                    name=f"rt{hb}")
                    psums = [wpools[wh].tile([112, ch[wh][1]], mybir.dt.float32,
                                             tag=f"ps{wh}", name=f"ps{wh}")
                             for wh in range(len(ch))]
                    for g in range(G):
                        # same mask for both fp8 planes (hi, lo)
                        lhs = strided(blob, [[BW, 128], [0, 2], [1, 112]],
                                      wtb + 224 * g + 112 * hb)
                        for wh in range(len(ch)):
                            # planes (hi, lo) of window hb at column shift 3g
                            rhs = strided(
                                blob, [[BW, 128], [2 * Wp, 2], [1, ch[wh][1]]],
                                hb * Wp + 3 * g + ch[wh][0])
                            nc.tensor.matmul(
                                psums[wh], lhsT=lhs, rhs=rhs,
                                start=(g == 0), stop=(g == G - 1),
                                perf_mode=mybir.MatmulPerfMode.DoubleRow)
                    for wh in range(len(ch)):
                        dstc = rt[:, ch[wh][0]:ch[wh][0] + ch[wh][1]]
                        if wh == 0:
                            nc.scalar.activation(
                                out=dstc, in_=psums[wh],
                                func=mybir.ActivationFunctionType.Copy)
                        else:
                            nc.vector.tensor_copy(out=dstc, in_=psums[wh])
                    for bd in range(nb):
                        u0 = int(meta["U0"][j, hb, bd])
                        bwd = int(meta["bw"][j, hb, bd])
                        ob = int(meta["obase"][j, hb, bd])
                        src = rt[bd * RB:(bd + 1) * RB, u0:u0 + bwd]
                        dst = out[0, ob:ob + RB * bwd]
                        eng = nc.scalar if (hb == 1 and bd == 1) else nc.gpsimd
                        eng.dma_start(out=dst, in_=src)
    return nc


def run_cores(meta, in_maps, trace=False):
    from concourse.bass_utils import run_bass_kernel_spmd

    nc = build_program(meta)
    nc.compile()
    dedupe_ldweights(nc)
    res = run_bass_kernel_spmd(nc, in_maps, core_ids=list(range(len(in_maps))),
                               trace=trace)
    return res


def unshard(meta, results):
    B = meta["mapping"].size
    nb = 112 // RB
    out = np.zeros((B, H, W, C), np.float32)
    for cidx, r in enumerate(results):
        o = np.asarray(r["out"], np.float32).reshape(-1)
        for j in range(meta["slots"]):
            b = meta["mapping"][cidx, j]
            img = np.zeros((H, WC), np.float32)
            for hb in range(2):
                for bd in range(nb):
                    u0 = int(meta["U0"][j, hb, bd])
                    bwd = int(meta["bw"][j, hb, bd])
                    ob = int(meta["obase"][j, hb, bd])
                    t = o[ob:ob + RB * bwd].reshape(RB, bwd)
                    om = meta["omega"][j, cidx, hb]
                    for rr in range(RB):
                        r_ = bd * RB + rr
                        u = 3 * int(om[r_]) - u0
                        img[112 * hb + r_] = t[rr, u:u + WC]
            img = img.reshape(H, W, C)
            if meta["transposed"][b]:
                img = img.transpose(1, 0, 2)
            out[b] = img
    return out


def kernel(x, kernels_table, amt, angles):
    x = np.asarray(x, np.float32)
    kernels_table = np.asarray(kernels_table, np.float32)
    amt = np.asarray(amt)
    angles = np.asarray(angles)
    meta, in_maps = prepare_host(x, kernels_table, amt, angles)
    res = run_cores(meta, in_maps)
    return unshard(meta, res.results)


# revision 14
# speedup vs baseline: 1.1472x; 1.1236x over previous
"""Trainium2 Bass kernel for nn_BlurLayer (B=128, 224x224x3, per-sample
rotated-line motion blur, SAME depthwise conv).

Self-contained: kernel(**inputs) -> np.ndarray. Shards the batch over 8
NeuronCores (pure data parallel: 16 samples per core), compiles + runs one
SPMD Bass program via concourse.bass_utils.run_bass_kernel_spmd, gathers
the full output.

Method: the rotated blur kernel's nonzero taps all equal 1/size and form a
digitized line. The conv is computed as G matmuls per 112-row output block:
out[r, u] += W_g[p, r] * window[p, u + 3g], where the window rows carry a
per-row horizontal shift sigma(m) and the psum columns a per-row output
shift omega(Y) (both baked host-side into the blob / unshard, so all
device access patterns are static). sigma/omega are chosen per sample by a
Bellman-Ford difference-constraint solver minimizing G (distinct shifted
column groups covering every tap) -- G lands in {1,2,3} for any angle
instead of the raw column span. Masks W_g are arbitrary per-sample 0/1
matrices (block-edge clipping implements vertical SAME padding). The scale
1/size = 2^-k * s' folds as 2^-k into the masks (exact in fp8) and s' into
the image, which is split into fp8e4m3 hi/lo planes so matmuls run in
DoubleRow perf mode and PSUM holds final values (plain copies). One DMA
per blob / per output half-block, ring-rotated across the three DMA queues
(sync/scalar/gpsimd) with a 3-slot input prefetch pipeline; junk warmup
matmuls hold the PE through the HAM ramp during the first DMAs; a
post-compile pass drops back-to-back reloads of identical PE weights.
"""

import math

import numpy as np
import ml_dtypes

MAXK = 32
H = W = 224
C = 3
WC = W * C                  # 672
PAD_LO = (MAXK - 1) // 2    # 15
PIMG_PAD = 800              # left zero margin (elems) of padded image row
PIMG_W = PIMG_PAD + WC + 800

FP8 = ml_dtypes.float8_e4m3


def dedupe_ldweights(nc):
    """Replace an InstLdweights whose weights AP is identical to the
    immediately-preceding one (per block) with an InstNoOp carrying its
    sync_info: the PE array still holds those weights, so the reload is
    pure Tensor-queue overhead."""
    import concourse.mybir as mybir
    n = 0
    for fn in nc.m.functions:
        for blk in fn.blocks:
            prev_key = None
            new_insts = []
            for inst in blk.instructions:
                if isinstance(inst, mybir.InstLdweights):
                    key = (repr(inst.ins), repr(getattr(inst, "perf_mode", None)),
                           repr(getattr(inst, "is_transpose", None)))
                    if key == prev_key:
                        n += 1
                        inst = mybir.InstNoOp(
                            name=f"{inst.name}-ldwdedup",
                            engine=inst.engine,
                            ins=[], outs=[],
                            sync_info=inst.sync_info,
                        )
                    else:
                        prev_key = key
                new_insts.append(inst)
            blk.instructions = new_insts
    return n


# ---------------------------------------------------------------- host math
def rotate_nearest_np(img, rad):
    K = img.shape[0]
    cos, sin = np.cos(rad), np.sin(rad)
    coords = np.arange(K, dtype=np.float32)
    yy, xx = np.meshgrid(coords, coords, indexing="ij")
    e = np.float32(K - 1)
    x_off = (e - (cos * e - sin * e)) * 0.5
    y_off = (e - (sin * e + cos * e)) * 0.5
    sx = cos * xx - sin * yy + x_off
    sy = sin * xx + cos * yy + y_off
    ix = np.round(sx).astype(np.int32)
    iy = np.round(sy).astype(np.int32)
    valid = (ix >= 0) & (ix < K) & (iy >= 0) & (iy < K)
    g = img[np.clip(iy, 0, K - 1), np.clip(ix, 0, K - 1)]
    return np.where(valid, g, np.float32(0.0))


def tap_pairs(ker):
    """Valid (m, Y, kx) triples: img row m = Y + ky - 15 for out row Y and
    tap (ky, kx), both m and Y in [0, 224)."""
    ys, xs = np.nonzero(ker)
    Y = np.arange(H)
    M, YY, KX = [], [], []
    for ky, kx in zip(ys, xs):
        m = Y + ky - 15
        ok = (m >= 0) & (m < H)
        M.append(m[ok])
        YY.append(Y[ok])
        KX.append(np.full(int(ok.sum()), kx))
    return np.concatenate(M), np.concatenate(YY), np.concatenate(KX)


def solve_shear(ker, max_rounds=400):
    """Choose integer shift profiles sigma (img rows) / nu (out rows)
    minimizing G = #values of g = kx - sigma[m] + nu[Y] over all taps.
    Difference-constraint feasibility via vectorized Bellman-Ford.
    Returns (G, sigma[224], nu[224], c) with g - c in [0, G)."""
    M, Y, KX = tap_pairs(ker)
    key = M * H + Y
    order = np.argsort(key)
    key_s, kx_s = key[order], KX[order]
    uk, idx = np.unique(key_s, return_index=True)
    lo = np.minimum.reduceat(kx_s, idx)
    hi = np.maximum.reduceat(kx_s, idx)
    um, uy = uk // H, uk % H
    G0 = int((hi - lo).max()) + 1
    for G in range(G0, 40):
        src = np.concatenate([224 + uy, um])
        dst = np.concatenate([um, 224 + uy])
        w = np.concatenate([lo, (G - 1) - hi]).astype(np.int64)
        dist = np.zeros(448, np.int64)
        ok = False
        for _ in range(max_rounds):
            nd = dist.copy()
            np.minimum.at(nd, dst, dist[src] + w)
            if np.array_equal(nd, dist):
                ok = True
                break
            dist = nd
        if ok:
            sigma, nu = dist[:224], dist[224:]
            g = KX - sigma[M] + nu[Y]
            c = int(g.min())
            assert int(g.max()) - c + 1 <= G
            return int(g.max()) - c + 1, sigma, nu, c
    raise RuntimeError("no feasible G")


def sample_plan(tbl_ch0, amt_b, ang_b):
    """-> dict with scale, G, sigma, nu, c, wout, transposed, ker."""
    rad = np.float32(ang_b * math.pi / 180.0)
    ker0 = rotate_nearest_np(tbl_ch0[amt_b], rad)
    ys, xs = np.nonzero(ker0)
    scale = float(ker0[ys[0], xs[0]])
    best = None
    for tr in (False, True):
        km = ker0.T if tr else ker0
        G, sigma, nu, c = solve_shear(km)
        w0 = int(nu[0:112].max() - nu[0:112].min())
        w1 = int(nu[112:224].max() - nu[112:224].min())
        wout = WC + 3 * max(w0, w1)
        cost = 2 * G * wout
        if best is None or cost < best["cost"]:
            best = dict(cost=cost, G=G, sigma=sigma, nu=nu, c=c, wout=wout,
                        transposed=tr, ker=km, scale=np.float32(scale))
    return best


def prepare_host(x, kernels_table, amt, angles, n_cores=8):
    B = x.shape[0]
    assert B % n_cores == 0
    slots = B // n_cores
    tbl_ch0 = np.ascontiguousarray(kernels_table[:, :, :, 0])

    plans = [sample_plan(tbl_ch0, int(amt[b]), int(angles[b]))
             for b in range(B)]
    Gs = np.array([p["G"] for p in plans])
    wos = np.array([p["wout"] for p in plans])

    # slot packing: G uniform-ish per slot (sort by G then wout, rows of 8)
    order = np.lexsort((-wos, -Gs))
    asg = order.reshape(slots, n_cores)

    slotG = np.array([Gs[asg[j]].max() for j in range(slots)])
    slotW = np.array([wos[asg[j]].max() for j in range(slots)])

    # schedule: a light slot first (its input DMA completes fastest, so
    # the PE starts early), the lightest last (small tail), heavy middle.
    slot_cost = 2 * slotG * slotW
    o = np.argsort(-slot_cost, kind="stable")   # heavy .. light
    sched = np.concatenate([[o[-2]], o[:-2], [o[-1]]])
    asg = asg[sched]
    slotG = slotG[sched]
    slotW = slotW[sched]

    gmax = slotG
    wout = slotW
    wprime = ((wout + 3 * gmax + 7) // 8) * 8
    blobw = 4 * wprime + 224 * gmax            # 4 image planes + mask table
    col_base = np.concatenate([[0], np.cumsum(blobw)])[:-1]
    totbw = int(blobw.sum())
    out_base = np.concatenate([[0], np.cumsum(2 * 112 * wout)])[:-1]
    totout = int((2 * 112 * wout).sum())

    in_maps = []
    mapping = np.zeros((n_cores, slots), np.int64)
    omega_all = np.zeros((n_cores, slots, 2, 112), np.int64)
    for cidx in range(n_cores):
        blob = np.zeros((128, totbw), FP8)
        for j in range(slots):
            b = int(asg[j, cidx])
            p = plans[b]
            G = int(gmax[j])
            Wp = int(wprime[j])
            base = int(col_base[j])
            mapping[cidx, j] = b
            sigma, nu, c = p["sigma"], p["nu"], p["c"]

            # fold 1/size = 2^-k * s' : 2^-k goes into the masks (exact in
            # fp8), s' in (0.5, 1] scales the image (no dynamic-range loss)
            size = int(round(1.0 / float(p["scale"])))
            k2 = int(math.floor(math.log2(size)))
            xs_ = x[b] * np.float32(2.0 ** k2 / size)
            if p["transposed"]:
                xs_ = xs_.transpose(1, 0, 2)
            xhi = xs_.astype(FP8)
            xlo = (xs_ - xhi.astype(np.float32)).astype(FP8)
            phi = np.zeros((H, PIMG_W), FP8)
            plo = np.zeros((H, PIMG_W), FP8)
            phi[:, PIMG_PAD:PIMG_PAD + WC] = xhi.reshape(H, WC)
            plo[:, PIMG_PAD:PIMG_PAD + WC] = xlo.reshape(H, WC)

            # blob layout: [hb0_hi | hb1_hi | hb0_lo | hb1_lo | masks]
            for hb, (R, S) in enumerate(((0, 0), (112, 96))):
                numax = int(nu[R:R + 112].max())
                omega_all[cidx, j, hb] = numax - nu[R:R + 112]
                V0 = PIMG_PAD + 3 * (c - PAD_LO - numax)
                rows = np.arange(128)
                cols = V0 + 3 * sigma[S + rows]
                assert cols.min() >= 0 and cols.max() + Wp <= PIMG_W, \
                    (b, hb, cols.min(), cols.max(), Wp)
                for pl, pimg in enumerate((phi, plo)):
                    dst = base + (2 * pl + hb) * Wp
                    win = np.zeros((128, Wp), FP8)
                    for pp in range(128):
                        win[pp] = pimg[S + pp, cols[pp]:cols[pp] + Wp]
                    blob[:, dst:dst + Wp] = win

            # mask table: [128, G, 2(hb), 112] fp8, value 2^-k
            wtb = base + 4 * Wp
            wcols = np.zeros((128, G, 2, 112), np.float32)
            ys, xs2 = np.nonzero(p["ker"])
            r = np.arange(112)
            for ky, kx in zip(ys, xs2):
                for hb, (R, S) in enumerate(((0, 0), (112, 96))):
                    m = R + r + ky - PAD_LO
                    pr = m - S
                    ok = (pr >= 0) & (pr < 128) & (m >= 0) & (m < H)
                    if not ok.any():
                        continue
                    g = (kx - sigma[m[ok]] + nu[R + r[ok]]) - c
                    assert g.min() >= 0 and g.max() < G, (b, hb, g.min(), g.max(), G)
                    wcols[pr[ok], g, hb, r[ok]] = 2.0 ** (-k2)
            blob[:, wtb:wtb + 224 * G] = np.ascontiguousarray(wcols).reshape(
                128, 224 * G).astype(FP8)
        in_maps.append({"ximg": blob})

    meta = {
        "slots": slots,
        "gmax": [int(v) for v in gmax],
        "wout": [int(v) for v in wout],
        "wprime": [int(v) for v in wprime],
        "blobw": [int(v) for v in blobw],
        "col_base": [int(v) for v in col_base],
        "out_base": [int(v) for v in out_base],
        "totbw": totbw,
        "totout": totout,
        "mapping": mapping,
        "omega": omega_all,
        "transposed": np.array([p["transposed"] for p in plans]),
    }
    return meta, in_maps


def _chunks(wout):
    """Split a result width into <=512-col PSUM chunks."""
    n = -(-wout // 512)
    w = -(-wout // n)
    out = []
    off = 0
    while off < wout:
        cc = min(w, wout - off)
        out.append((off, cc))
        off += cc
    return out


# ---------------------------------------------------------------- device IR
def build_program(meta):
    import concourse.bacc as bacc
    import concourse.mybir as mybir
    from concourse.tile import TileContext
    from bass_rust import VecI64Pair

    fp8 = mybir.dt.float8e4
    slots = meta["slots"]

    nc = bacc.Bacc("TRN2")
    ximg = nc.dram_tensor("ximg", [128, meta["totbw"]], fp8, kind="ExternalInput")
    out = nc.dram_tensor("out", [1, meta["totout"]], mybir.dt.float16,
                         kind="ExternalOutput")

    def strided(tile, dims, offset):
        ap = tile[:, 0:1].copy()
        ap.ap = VecI64Pair(dims)
        ap.offset = offset
        return ap

    with TileContext(nc) as tc:
        with tc.tile_pool(name="img", bufs=slots) as ipool, \
             tc.tile_pool(name="res", bufs=10) as rpool, \
             tc.tile_pool(name="warm", bufs=1) as wmpool, \
             tc.tile_pool(name="ps0", bufs=2, space="PSUM") as pw0, \
             tc.tile_pool(name="ps1", bufs=2, space="PSUM") as pw1, \
             tc.tile_pool(name="psw", bufs=1, space="PSUM") as pww:

            # PE warmup: ~7 junk matmuls keep the PE busy through the HAM
            # activity window while the first blobs stream in, so the real
            # matmuls start at 2.4 GHz instead of 1.2.
            wm = wmpool.tile([128, 624], fp8)
            nc.vector.memset(wm, 0.0)
            wps = pww.tile([112, 512], mybir.dt.float32)
            for _ in range(7):
                nc.tensor.matmul(wps, lhsT=wm[:, 0:112], rhs=wm[:, 112:624],
                                 start=True, stop=True)

            wpools = [pw0, pw1]
            blobs = [None] * slots

            def prefetch(j):
                # schedule all input DMAs on sync (any other assignment
                # perturbs the tile scheduler's matmul ordering and breaks
                # ldweights dedupe); after compile, reassign_input_dmas()
                # flips alternate ones to the scalar queue for a 2nd ring.
                BW = meta["blobw"][j]
                base = meta["col_base"][j]
                blob = ipool.tile([128, BW], fp8, tag="blob", name="blob")
                nc.sync.dma_start(out=blob, in_=ximg[:, base:base + BW])
                blobs[j] = blob

            PF = 3                          # slots of input lookahead
            for j in range(min(PF, slots)):
                prefetch(j)

            for j in range(slots):
                if j + PF < slots:
                    prefetch(j + PF)
                G = meta["gmax"][j]
                WO = meta["wout"][j]
                Wp = meta["wprime"][j]
                BW = meta["blobw"][j]
                obase = meta["out_base"][j]
                ch = _chunks(WO)
                assert len(ch) <= 2
                blob = blobs[j]
                wtb = 4 * Wp

                for hb in (0, 1):
                    rt = rpool.tile([112, WO], mybir.dt.float16, tag=f"rt{hb}",
                                    name=f"rt{hb}")
                    psums = [wpools[wh].tile([112, ch[wh][1]], mybir.dt.float32,
                                             tag=f"ps{wh}", name=f"ps{wh}")
                             for wh in range(len(ch))]
                    for g in range(G):
                        # same mask for both fp8 planes (hi, lo)
                        lhs = strided(blob, [[BW, 128], [0, 2], [1, 112]],
                                      wtb + 224 * g + 112 * hb)
                        for wh in range(len(ch)):
                            # planes (hi, lo) of window hb at column shift 3g
                            rhs = strided(
                                blob, [[BW, 128], [2 * Wp, 2], [1, ch[wh][1]]],
                                hb * Wp + 3 * g + ch[wh][0])
                            nc.tensor.matmul(
                                psums[wh], lhsT=lhs, rhs=rhs,
                                start=(g == 0), stop=(g == G - 1),
                                perf_mode=mybir.MatmulPerfMode.DoubleRow)
                    for wh in range(len(ch)):
                        dstc = rt[:, ch[wh][0]:ch[wh][0] + ch[wh][1]]
                        if wh == 0:
                            nc.scalar.activation(
                                out=dstc, in_=psums[wh],
                                func=mybir.ActivationFunctionType.Copy)
                        else:
                            nc.vector.tensor_copy(out=dstc, in_=psums[wh])
                    # output DMA per half-block; rotate rings to spread load
                    src = strided(rt, [[WO, 112], [1, WO]], 0)
                    dst = out[0, 0:1].copy()
                    dst.ap = VecI64Pair([[WO, 112], [1, WO]])
                    dst.offset = obase + hb * 112 * WO
                    if hb == 0:
                        eng = nc.gpsimd
                    else:
                        eng = nc.gpsimd if j % 2 == 0 else nc.sync
                    eng.dma_start(out=dst, in_=src)
    return nc


def reassign_input_dmas(nc):
    """Post-compile: move every other input-blob DMA (sync queue, SBUF dst)
    to the scalar queue so the input streams over two DMA rings. Input DMAs
    carry no waits (one pool buffer per slot), so queue reassignment cannot
    deadlock; semaphores are global and move with the instruction."""
    import concourse.mybir as mybir
    moved = 0
    seen = 0
    for fn in nc.m.functions:
        for blk in fn.blocks:
            for inst in blk.instructions:
                if not isinstance(inst, mybir.InstDMACopy):
                    continue
                if inst.engine != mybir.EngineType.SP:
                    continue
                if not str(getattr(inst.outs[0], "memref", "")).startswith("blob"):
                    continue
                seen += 1
                if seen % 2 == 0:
                    inst.engine = mybir.EngineType.Activation
                    moved += 1
    return moved


def run_cores(meta, in_maps, trace=False):
    from concourse.bass_utils import run_bass_kernel_spmd

    nc = build_program(meta)
    nc.compile()
    dedupe_ldweights(nc)
    reassign_input_dmas(nc)
    res = run_bass_kernel_spmd(nc, in_maps, core_ids=list(range(len(in_maps))),
                               trace=trace)
    return res


def unshard(meta, results):
    B = meta["mapping"].size
    out = np.zeros((B, H, W, C), np.float32)
    for cidx, r in enumerate(results):
        o = np.asarray(r["out"], np.float32).reshape(-1)
        for j in range(meta["slots"]):
            b = meta["mapping"][cidx, j]
            WO = meta["wout"][j]
            t = o[meta["out_base"][j]:meta["out_base"][j] + 2 * 112 * WO]
            t = t.reshape(2, 112, WO)
            img = np.zeros((H, WC), np.float32)
            om = meta["omega"][cidx, j]
            for hb in (0, 1):
                for r_ in range(112):
                    u = 3 * int(om[hb, r_])
                    img[112 * hb + r_] = t[hb, r_, u:u + WC]
            img = img.reshape(H, W, C)
            if meta["transposed"][b]:
                img = img.transpose(1, 0, 2)
            out[b] = img
    return out


def kernel(x, kernels_table, amt, angles):
    x = np.asarray(x, np.float32)
    kernels_table = np.asarray(kernels_table, np.float32)
    amt = np.asarray(amt)
    angles = np.asarray(angles)
    meta, in_maps = prepare_host(x, kernels_table, amt, angles)
    res = run_cores(meta, in_maps)
    return unshard(meta, res.results)


# revision 19
# speedup vs baseline: 1.1816x; 1.0300x over previous
"""Trainium2 Bass kernel for nn_BlurLayer (B=128, 224x224x3, per-sample
rotated-line motion blur, SAME depthwise conv).

Self-contained: kernel(**inputs) -> np.ndarray. Shards the batch over 8
NeuronCores (pure data parallel: 16 samples per core), compiles + runs one
SPMD Bass program via concourse.bass_utils.run_bass_kernel_spmd, gathers
the full output.

Method: the rotated blur kernel's nonzero taps all equal 1/size and form a
digitized line. The conv is computed as G matmuls per 112-row output block:
out[r, u] += W_g[p, r] * window[p, u + 3g], where the window rows carry a
per-row horizontal shift sigma(m) and the psum columns a per-row output
shift omega(Y) (both baked host-side into the blob / unshard, so all
device access patterns are static). sigma/omega are chosen per sample by a
Bellman-Ford difference-constraint solver minimizing G (distinct shifted
column groups covering every tap) -- G lands in {1,2,3} for any angle
instead of the raw column span. Masks W_g are arbitrary per-sample 0/1
matrices (block-edge clipping implements vertical SAME padding). The scale
1/size = 2^-k * s' folds as 2^-k into the masks (exact in fp8) and s' into
the image, which is split into fp8e4m3 hi/lo planes so matmuls run in
DoubleRow perf mode and PSUM holds final values (plain copies). One DMA
per blob / per output half-block, ring-rotated across the three DMA queues
(sync/scalar/gpsimd) with a 3-slot input prefetch pipeline; junk warmup
matmuls hold the PE through the HAM ramp during the first DMAs; a
post-compile pass drops back-to-back reloads of identical PE weights.
"""

import math

import numpy as np
import ml_dtypes

MAXK = 32
H = W = 224
C = 3
WC = W * C                  # 672
PAD_LO = (MAXK - 1) // 2    # 15
PIMG_PAD = 800              # left zero margin (elems) of padded image row
PIMG_W = PIMG_PAD + WC + 800

FP8 = ml_dtypes.float8_e4m3
BF16 = ml_dtypes.bfloat16


def dedupe_ldweights(nc):
    """Replace an InstLdweights whose weights AP is identical to the
    immediately-preceding one (per block) with an InstNoOp carrying its
    sync_info: the PE array still holds those weights, so the reload is
    pure Tensor-queue overhead."""
    import concourse.mybir as mybir
    n = 0
    for fn in nc.m.functions:
        for blk in fn.blocks:
            prev_key = None
            new_insts = []
            for inst in blk.instructions:
                if isinstance(inst, mybir.InstLdweights):
                    key = (repr(inst.ins), repr(getattr(inst, "perf_mode", None)),
                           repr(getattr(inst, "is_transpose", None)))
                    if key == prev_key:
                        n += 1
                        inst = mybir.InstNoOp(
                            name=f"{inst.name}-ldwdedup",
                            engine=inst.engine,
                            ins=[], outs=[],
                            sync_info=inst.sync_info,
                        )
                    else:
                        prev_key = key
                new_insts.append(inst)
            blk.instructions = new_insts
    return n


# ---------------------------------------------------------------- host math
def rotate_nearest_np(img, rad):
    K = img.shape[0]
    cos, sin = np.cos(rad), np.sin(rad)
    coords = np.arange(K, dtype=np.float32)
    yy, xx = np.meshgrid(coords, coords, indexing="ij")
    e = np.float32(K - 1)
    x_off = (e - (cos * e - sin * e)) * 0.5
    y_off = (e - (sin * e + cos * e)) * 0.5
    sx = cos * xx - sin * yy + x_off
    sy = sin * xx + cos * yy + y_off
    ix = np.round(sx).astype(np.int32)
    iy = np.round(sy).astype(np.int32)
    valid = (ix >= 0) & (ix < K) & (iy >= 0) & (iy < K)
    g = img[np.clip(iy, 0, K - 1), np.clip(ix, 0, K - 1)]
    return np.where(valid, g, np.float32(0.0))


def tap_pairs(ker):
    """Valid (m, Y, kx) triples: img row m = Y + ky - 15 for out row Y and
    tap (ky, kx), both m and Y in [0, 224)."""
    ys, xs = np.nonzero(ker)
    Y = np.arange(H)
    M, YY, KX = [], [], []
    for ky, kx in zip(ys, xs):
        m = Y + ky - 15
        ok = (m >= 0) & (m < H)
        M.append(m[ok])
        YY.append(Y[ok])
        KX.append(np.full(int(ok.sum()), kx))
    return np.concatenate(M), np.concatenate(YY), np.concatenate(KX)


def solve_shear(ker, max_rounds=400):
    """Choose integer shift profiles sigma (img rows) / nu (out rows)
    minimizing G = #values of g = kx - sigma[m] + nu[Y] over all taps.
    Difference-constraint feasibility via vectorized Bellman-Ford.
    Returns (G, sigma[224], nu[224], c) with g - c in [0, G)."""
    M, Y, KX = tap_pairs(ker)
    key = M * H + Y
    order = np.argsort(key)
    key_s, kx_s = key[order], KX[order]
    uk, idx = np.unique(key_s, return_index=True)
    lo = np.minimum.reduceat(kx_s, idx)
    hi = np.maximum.reduceat(kx_s, idx)
    um, uy = uk // H, uk % H
    G0 = int((hi - lo).max()) + 1
    for G in range(G0, 40):
        src = np.concatenate([224 + uy, um])
        dst = np.concatenate([um, 224 + uy])
        w = np.concatenate([lo, (G - 1) - hi]).astype(np.int64)
        dist = np.zeros(448, np.int64)
        ok = False
        for _ in range(max_rounds):
            nd = dist.copy()
            np.minimum.at(nd, dst, dist[src] + w)
            if np.array_equal(nd, dist):
                ok = True
                break
            dist = nd
        if ok:
            sigma, nu = dist[:224], dist[224:]
            g = KX - sigma[M] + nu[Y]
            c = int(g.min())
            assert int(g.max()) - c + 1 <= G
            return int(g.max()) - c + 1, sigma, nu, c
    raise RuntimeError("no feasible G")


def sample_plan(tbl_ch0, amt_b, ang_b):
    """-> dict with scale, G, sigma, nu, c, wout, transposed, ker."""
    rad = np.float32(ang_b * math.pi / 180.0)
    ker0 = rotate_nearest_np(tbl_ch0[amt_b], rad)
    ys, xs = np.nonzero(ker0)
    scale = float(ker0[ys[0], xs[0]])
    best = None
    for tr in (False, True):
        km = ker0.T if tr else ker0
        G, sigma, nu, c = solve_shear(km)
        w0 = int(nu[0:112].max() - nu[0:112].min())
        w1 = int(nu[112:224].max() - nu[112:224].min())
        wout = WC + 3 * max(w0, w1)
        cost = 2 * G * wout
        if best is None or cost < best["cost"]:
            best = dict(cost=cost, G=G, sigma=sigma, nu=nu, c=c, wout=wout,
                        transposed=tr, ker=km, scale=np.float32(scale))
    return best


def prepare_host(x, kernels_table, amt, angles, n_cores=8):
    B = x.shape[0]
    assert B % n_cores == 0
    slots = B // n_cores
    tbl_ch0 = np.ascontiguousarray(kernels_table[:, :, :, 0])

    plans = [sample_plan(tbl_ch0, int(amt[b]), int(angles[b]))
             for b in range(B)]
    Gs = np.array([p["G"] for p in plans])
    wos = np.array([p["wout"] for p in plans])

    # slot packing: G uniform-ish per slot (sort by G then wout, rows of 8)
    order = np.lexsort((-wos, -Gs))
    asg = order.reshape(slots, n_cores)

    slotG = np.array([Gs[asg[j]].max() for j in range(slots)])
    slotW = np.array([wos[asg[j]].max() for j in range(slots)])

    # schedule: a light slot first (its input DMA completes fastest, so
    # the PE starts early), the lightest last (small tail), heavy middle.
    slot_cost = 2 * slotG * slotW
    o = np.argsort(-slot_cost, kind="stable")   # heavy .. light
    sched = np.concatenate([[o[-2]], o[:-2], [o[-1]]])
    asg = asg[sched]
    slotG = slotG[sched]
    slotW = slotW[sched]

    gmax = slotG
    wout = slotW
    wprime = ((wout + 3 * gmax + 7) // 8) * 8
    blobw = 4 * wprime + 224 * gmax            # 4 image planes + mask table
    col_base = np.concatenate([[0], np.cumsum(blobw)])[:-1]
    totbw = int(blobw.sum())
    out_base = np.concatenate([[0], np.cumsum(2 * 112 * wout)])[:-1]
    totout = int((2 * 112 * wout).sum())

    in_maps = []
    mapping = np.zeros((n_cores, slots), np.int64)
    omega_all = np.zeros((n_cores, slots, 2, 112), np.int64)
    for cidx in range(n_cores):
        blob = np.zeros((128, totbw), np.uint8)
        for j in range(slots):
            b = int(asg[j, cidx])
            p = plans[b]
            G = int(gmax[j])
            Wp = int(wprime[j])
            base = int(col_base[j])
            mapping[cidx, j] = b
            sigma, nu, c = p["sigma"], p["nu"], p["c"]

            # fold 1/size = 2^-k * s' : 2^-k goes into the masks (exact in
            # fp8), s' in (0.5, 1] scales the image (no dynamic-range loss)
            size = int(round(1.0 / float(p["scale"])))
            k2 = int(math.floor(math.log2(size)))
            xs_ = x[b] * np.float32(2.0 ** k2 / size)
            if p["transposed"]:
                xs_ = xs_.transpose(1, 0, 2)
            pimg = np.zeros((H, PIMG_W), BF16)
            pimg[:, PIMG_PAD:PIMG_PAD + WC] = xs_.reshape(H, WC).astype(BF16)

            # blob bytes layout: [hb0 bf16 | hb1 bf16 | masks fp8]
            for hb, (R, S) in enumerate(((0, 0), (112, 96))):
                numax = int(nu[R:R + 112].max())
                omega_all[cidx, j, hb] = numax - nu[R:R + 112]
                V0 = PIMG_PAD + 3 * (c - PAD_LO - numax)
                rows = np.arange(128)
                cols = V0 + 3 * sigma[S + rows]
                assert cols.min() >= 0 and cols.max() + Wp <= PIMG_W, \
                    (b, hb, cols.min(), cols.max(), Wp)
                win = np.zeros((128, Wp), BF16)
                for pp in range(128):
                    win[pp] = pimg[S + pp, cols[pp]:cols[pp] + Wp]
                dst = base + 2 * hb * Wp
                blob[:, dst:dst + 2 * Wp] = win.view(np.uint8)

            # mask table: [128, G, 2(hb), 112] fp8, value 2^-k
            wtb = base + 4 * Wp
            wcols = np.zeros((128, G, 2, 112), np.float32)
            ys, xs2 = np.nonzero(p["ker"])
            r = np.arange(112)
            for ky, kx in zip(ys, xs2):
                for hb, (R, S) in enumerate(((0, 0), (112, 96))):
                    m = R + r + ky - PAD_LO
                    pr = m - S
                    ok = (pr >= 0) & (pr < 128) & (m >= 0) & (m < H)
                    if not ok.any():
                        continue
                    g = (kx - sigma[m[ok]] + nu[R + r[ok]]) - c
                    assert g.min() >= 0 and g.max() < G, (b, hb, g.min(), g.max(), G)
                    wcols[pr[ok], g, hb, r[ok]] = 2.0 ** (-k2)
            blob[:, wtb:wtb + 224 * G] = np.ascontiguousarray(wcols).reshape(
                128, 224 * G).astype(FP8).view(np.uint8)
        in_maps.append({"ximg": blob.view(FP8)})

    meta = {
        "slots": slots,
        "gmax": [int(v) for v in gmax],
        "wout": [int(v) for v in wout],
        "wprime": [int(v) for v in wprime],
        "blobw": [int(v) for v in blobw],
        "col_base": [int(v) for v in col_base],
        "out_base": [int(v) for v in out_base],
        "totbw": totbw,
        "totout": totout,
        "mapping": mapping,
        "omega": omega_all,
        "transposed": np.array([p["transposed"] for p in plans]),
    }
    return meta, in_maps


def _chunks(wout):
    """Split a result width into <=512-col PSUM chunks."""
    n = -(-wout // 512)
    w = -(-wout // n)
    out = []
    off = 0
    while off < wout:
        cc = min(w, wout - off)
        out.append((off, cc))
        off += cc
    return out


# ---------------------------------------------------------------- device IR
def build_program(meta):
    import concourse.bacc as bacc
    import concourse.mybir as mybir
    from concourse.tile import TileContext
    from bass_rust import VecI64Pair

    fp8 = mybir.dt.float8e4
    slots = meta["slots"]

    nc = bacc.Bacc("TRN2")
    ximg = nc.dram_tensor("ximg", [128, meta["totbw"]], fp8, kind="ExternalInput")
    out = nc.dram_tensor("out", [1, meta["totout"]], mybir.dt.float16,
                         kind="ExternalOutput")

    def strided(tile, dims, offset):
        ap = tile[:, 0:1].copy()
        ap.ap = VecI64Pair(dims)
        ap.offset = offset
        return ap

    with TileContext(nc) as tc:
        with tc.tile_pool(name="img", bufs=slots) as ipool, \
             tc.tile_pool(name="res", bufs=10) as rpool, \
             tc.tile_pool(name="warm", bufs=1) as wmpool, \
             tc.tile_pool(name="ps0", bufs=2, space="PSUM") as pw0, \
             tc.tile_pool(name="ps1", bufs=2, space="PSUM") as pw1, \
             tc.tile_pool(name="psw", bufs=1, space="PSUM") as pww:

            # PE warmup: ~7 junk matmuls keep the PE busy through the HAM
            # activity window while the first blobs stream in, so the real
            # matmuls start at 2.4 GHz instead of 1.2.
            wm = wmpool.tile([128, 624], fp8)
            nc.vector.memset(wm, 0.0)
            wps = pww.tile([112, 512], mybir.dt.float32)
            for _ in range(7):
                nc.tensor.matmul(wps, lhsT=wm[:, 0:112], rhs=wm[:, 112:624],
                                 start=True, stop=True)

            wpools = [pw0, pw1]
            blobs = [None] * slots

            def prefetch(j):
                # schedule all input DMAs on sync (any other assignment
                # perturbs the tile scheduler's matmul ordering and breaks
                # ldweights dedupe); after compile, reassign_input_dmas()
                # flips alternate ones to the scalar queue for a 2nd ring.
                BW = meta["blobw"][j]
                base = meta["col_base"][j]
                blob = ipool.tile([128, BW], fp8, tag="blob", name="blob")
                nc.sync.dma_start(out=blob, in_=ximg[:, base:base + BW])
                blobs[j] = blob

            PF = 3                          # slots of input lookahead
            for j in range(min(PF, slots)):
                prefetch(j)

            for j in range(slots):
                if j + PF < slots:
                    prefetch(j + PF)
                G = meta["gmax"][j]
                WO = meta["wout"][j]
                Wp = meta["wprime"][j]
                BW = meta["blobw"][j]
                obase = meta["out_base"][j]
                ch = _chunks(WO)
                assert len(ch) <= 2
                blob = blobs[j]
                wtb = 4 * Wp

                for hb in (0, 1):
                    rt = rpool.tile([112, WO], mybir.dt.float16, tag=f"rt{hb}",
                                    name=f"rt{hb}")
                    psums = [wpools[wh].tile([112, ch[wh][1]], mybir.dt.float32,
                                             tag=f"ps{wh}", name=f"ps{wh}")
                             for wh in range(len(ch))]
                    for g in range(G):
                        # fp8 mask (stationary) x bf16 window view (moving)
                        lhs = strided(blob, [[BW, 128], [1, 112]],
                                      wtb + 224 * g + 112 * hb)
                        for wh in range(len(ch)):
                            rhs = blob[:, 0:2].bitcast(mybir.dt.bfloat16).copy()
                            rhs.ap = VecI64Pair([[BW // 2, 128],
                                                 [1, ch[wh][1]]])
                            rhs.offset = hb * Wp + 3 * g + ch[wh][0]
                            nc.tensor.matmul(
                                psums[wh], lhsT=lhs, rhs=rhs,
                                start=(g == 0), stop=(g == G - 1))
                    for wh in range(len(ch)):
                        dstc = rt[:, ch[wh][0]:ch[wh][0] + ch[wh][1]]
                        if wh == 0:
                            nc.scalar.activation(
                                out=dstc, in_=psums[wh],
                                func=mybir.ActivationFunctionType.Copy)
                        else:
                            nc.vector.tensor_copy(out=dstc, in_=psums[wh])
                    # output DMA per half-block; rotate rings to spread load
                    src = strided(rt, [[WO, 112], [1, WO]], 0)
                    dst = out[0, 0:1].copy()
                    dst.ap = VecI64Pair([[WO, 112], [1, WO]])
                    dst.offset = obase + hb * 112 * WO
                    if hb == 0:
                        eng = nc.gpsimd
                    else:
                        eng = nc.gpsimd if j % 2 == 0 else nc.sync
                    eng.dma_start(out=dst, in_=src)
    return nc


def reassign_input_dmas(nc):
    """Post-compile: move every other input-blob DMA (sync queue, SBUF dst)
    to the scalar queue so the input streams over two DMA rings. Input DMAs
    carry no waits (one pool buffer per slot), so queue reassignment cannot
    deadlock; semaphores are global and move with the instruction."""
    import concourse.mybir as mybir
    moved = 0
    seen = 0
    for fn in nc.m.functions:
        for blk in fn.blocks:
            for inst in blk.instructions:
                if not isinstance(inst, mybir.InstDMACopy):
                    continue
                if inst.engine != mybir.EngineType.SP:
                    continue
                if not str(getattr(inst.outs[0], "memref", "")).startswith("blob"):
                    continue
                seen += 1
                if seen % 2 == 0:
                    inst.engine = mybir.EngineType.Activation
                    moved += 1
    return moved


def run_cores(meta, in_maps, trace=False):
    from concourse.bass_utils import run_bass_kernel_spmd

    nc = build_program(meta)
    nc.compile()
    dedupe_ldweights(nc)
    reassign_input_dmas(nc)
    res = run_bass_kernel_spmd(nc, in_maps, core_ids=list(range(len(in_maps))),
                               trace=trace)
    return res


def unshard(meta, results):
    B = meta["mapping"].size
    out = np.zeros((B, H, W, C), np.float32)
    for cidx, r in enumerate(results):
        o = np.asarray(r["out"], np.float32).reshape(-1)
        for j in range(meta["slots"]):
            b = meta["mapping"][cidx, j]
            WO = meta["wout"][j]
            t = o[meta["out_base"][j]:meta["out_base"][j] + 2 * 112 * WO]
            t = t.reshape(2, 112, WO)
            img = np.zeros((H, WC), np.float32)
            om = meta["omega"][cidx, j]
            for hb in (0, 1):
                for r_ in range(112):
                    u = 3 * int(om[hb, r_])
                    img[112 * hb + r_] = t[hb, r_, u:u + WC]
            img = img.reshape(H, W, C)
            if meta["transposed"][b]:
                img = img.transpose(1, 0, 2)
            out[b] = img
    return out


def kernel(x, kernels_table, amt, angles):
    x = np.asarray(x, np.float32)
    kernels_table = np.asarray(kernels_table, np.float32)
    amt = np.asarray(amt)
    angles = np.asarray(angles)
    meta, in_maps = prepare_host(x, kernels_table, amt, angles)
    res = run_cores(meta, in_maps)
    return unshard(meta, res.results)
